# revision 5
# baseline (speedup 1.0000x reference)
"""Trainium2 Bass kernel for CenterWoParamMultiCosineLoss (l2Norm branch).

Contract: kernel(**inputs) takes FULL inputs (x [8192,1024] f32,
labels [8192] i64, centers [90,16,1024] f32) and returns the FULL output
(scalar f32 loss), running on 8 NeuronCores data-parallel over the batch.

Math (per sample b, with label c = labels[b], K=16 centers per class):
    xn = x / ||x||;  cn = centers / ||centers||  (rows, +1e-12 under sqrt)
    t_k = xn . cn[c,k]                (16 cosine sims)
    d_k = 1 - t_k
    per_sample = sum_k (1 - d_k/sd) * d_k = sd - ssq/sd
      where sd = sum_k d_k = 16 - T,  ssq = sum_k d_k^2 = 16 - 2T + Q,
            T = sum_k t_k,  Q = sum_k t_k^2
    loss = mean(per_sample)

Device strategy per core (1024 samples):
    - S[b, ck] = x_bf16 @ CnT_bf16 for ALL 1440 (class,k) columns (PE).
    - masked = S * onehot(label-per-column); exactly one class block per row
      is nonzero so T_raw = rowsum(masked), Q_raw = rowsum(masked^2) are plain
      full-row reductions (ACT accum_out).
    - x is NOT pre-normalized: T = T_raw/||x||, Q = Q_raw/||x||^2 in the tail.
    - Host sums the 8x[128,8] per-sample values -> mean.
"""

import os
import sys
from contextlib import ExitStack

import numpy as np

for _p in ("/opt/trn_rl_repo", "/root/.axon_site/_ro/trn_rl_repo"):
    if os.path.isdir(_p) and _p not in sys.path:
        sys.path.insert(0, _p)

import ml_dtypes

import concourse.bacc as bacc
import concourse.tile as tile
from concourse import bass_utils, mybir

N_CORES = 8
B_LOCAL = 1024          # samples per core
P = 128                 # partitions
N_TILES = B_LOCAL // P  # 8 sample tiles per core
D = 1024                # feature dim
C = 90                  # classes
K = 16                  # centers per class
CK = C * K              # 1440
D_CHUNKS = D // P       # 8 contraction chunks
EPS = 1e-12

FP32 = mybir.dt.float32
BF16 = mybir.dt.bfloat16

_NC_CACHE = {}


def _build_nc():
    nc = bacc.Bacc("TRN2", target_bir_lowering=False, debug=False)

    x_dram = nc.dram_tensor("x", [B_LOCAL, D], FP32, kind="ExternalInput").ap()
    labels_dram = nc.dram_tensor("labels", [P, N_TILES], FP32, kind="ExternalInput").ap()
    centers_dram = nc.dram_tensor("centers", [CK, D], FP32, kind="ExternalInput").ap()
    ident_dram = nc.dram_tensor("ident", [P, P], BF16, kind="ExternalInput").ap()
    colck_dram = nc.dram_tensor("colck", [P, CK], BF16, kind="ExternalInput").ap()
    out_dram = nc.dram_tensor("out", [P, N_TILES], FP32, kind="ExternalOutput").ap()

    with tile.TileContext(nc) as tc, ExitStack() as ctx:
        singles = ctx.enter_context(tc.tile_pool(name="singles", bufs=1))
        cpool = ctx.enter_context(tc.tile_pool(name="cpool", bufs=2))
        xpool = ctx.enter_context(tc.tile_pool(name="xpool", bufs=3))
        spool = ctx.enter_context(tc.tile_pool(name="spool", bufs=2))
        psum = ctx.enter_context(tc.tile_pool(name="psum", bufs=2, space="PSUM"))

        # ---- constants (host-provided) ----
        ident = singles.tile([P, P], BF16, tag="ident")
        nc.sync.dma_start(out=ident, in_=ident_dram)
        colck = singles.tile([P, CK], BF16, tag="colck")  # class id per S column
        nc.sync.dma_start(out=colck, in_=colck_dram)
        eps_col = singles.tile([P, 1], FP32, tag="eps_col")
        nc.vector.memset(eps_col, EPS)

        # labels for all 8 sample tiles: [128, 8]
        labels_sb = singles.tile([P, N_TILES], FP32, tag="labels_sb")
        nc.sync.dma_start(out=labels_sb, in_=labels_dram)

        # persistent transposed-normalized centers:
        # CnT_sb[p, j*CK + ck] = cn_bf[ck, j*128 + p]
        cnt_sb = singles.tile([P, D_CHUNKS * CK], BF16, tag="cnt_sb")
        cnt_view = cnt_sb.rearrange("p (j n) -> p j n", j=D_CHUNKS)

        # per-sample stats accumulated across tiles
        ss_all = singles.tile([P, N_TILES], FP32, tag="ss_all")  # sum x^2
        t_all = singles.tile([P, N_TILES], FP32, tag="t_all")    # T_raw
        q_all = singles.tile([P, N_TILES], FP32, tag="q_all")    # Q_raw

        # scratch for ACT accumulate outs (value unused)
        junk_f32 = singles.tile([P, D], FP32, tag="junk_f32")
        junk_bf = singles.tile([P, CK], BF16, tag="junk_bf")

        # ---- phase A: centers -> normalized bf16, transposed ----
        # 12 row-tiles: 11 x 128 rows + 1 x 32 rows (128 rows = 8 whole classes)
        n_ctiles = (CK + P - 1) // P
        for ct in range(n_ctiles):
            r0 = ct * P
            rn = min(P, CK - r0)
            c_t = cpool.tile([P, D], FP32, tag="c_t")
            nc.sync.dma_start(out=c_t[:rn], in_=centers_dram[r0:r0 + rn, :])
            ss_c = cpool.tile([P, 1], FP32, tag="ss_c")
            nc.scalar.activation(out=junk_f32[:rn], in_=c_t[:rn],
                                 func=mybir.ActivationFunctionType.Square,
                                 accum_out=ss_c[:rn])
            nc.scalar.activation(out=ss_c[:rn], in_=ss_c[:rn],
                                 func=mybir.ActivationFunctionType.Sqrt,
                                 bias=eps_col[:rn])
            rinv_c = cpool.tile([P, 1], FP32, tag="rinv_c")
            nc.vector.reciprocal(out=rinv_c[:rn], in_=ss_c[:rn])
            cn_bf = cpool.tile([P, D], BF16, tag="cn_bf")
            nc.vector.tensor_scalar_mul(cn_bf[:rn], c_t[:rn], rinv_c[:rn])

            # transpose rn x 128 blocks -> psum [128, 8*rn] bf16 (one bank)
            pt = psum.tile([P, D_CHUNKS * P], BF16, tag="pt")
            for j in range(D_CHUNKS):
                nc.tensor.transpose(pt[:, j * rn:(j + 1) * rn],
                                    cn_bf[:rn, j * P:(j + 1) * P], ident[:rn, :rn])
            # one strided copyback into the 8 d-chunk segments
            src = pt[:, :D_CHUNKS * rn].rearrange("p (j n) -> p j n", j=D_CHUNKS)
            nc.vector.tensor_copy(cnt_view[:, :, r0:r0 + rn], src)

        # ---- phase B: per 128-sample tile ----
        for t in range(N_TILES):
            x_t = xpool.tile([P, D], FP32, tag="x_t")
            nc.sync.dma_start(out=x_t, in_=x_dram[t * P:(t + 1) * P, :])

            # ss = sum x^2 (fp32)
            nc.scalar.activation(out=junk_f32, in_=x_t,
                                 func=mybir.ActivationFunctionType.Square,
                                 accum_out=ss_all[:, t:t + 1])
            # cast to bf16 (unnormalized)
            x_bf = xpool.tile([P, D], BF16, tag="x_bf")
            nc.scalar.activation(out=x_bf, in_=x_t,
                                 func=mybir.ActivationFunctionType.Copy)

            # transpose x_bf -> xT_sb[p, j*128 + b] = x_bf[b, j*128+p]
            pt = psum.tile([P, D_CHUNKS * P], BF16, tag="pt")
            for j in range(D_CHUNKS):
                nc.tensor.transpose(pt[:, j * P:(j + 1) * P],
                                    x_bf[:, j * P:(j + 1) * P], ident)
            xt_sb = xpool.tile([P, D], BF16, tag="xt_sb")
            nc.vector.tensor_copy(xt_sb, pt)

            # S[b, ck] = sum_d x[b,d] cn[ck,d] : accumulate 8 d-chunks
            s_ps = psum.tile([P, CK], FP32, tag="s_ps")
            n_slices = [(0, 512), (512, 512), (1024, CK - 1024)]
            for j in range(D_CHUNKS):
                lhsT = xt_sb[:, j * P:(j + 1) * P]
                for (n0, nw) in n_slices:
                    nc.tensor.matmul(s_ps[:, n0:n0 + nw], lhsT,
                                     cnt_sb[:, j * CK + n0: j * CK + n0 + nw],
                                     start=(j == 0), stop=(j == D_CHUNKS - 1))

            # one-hot over all 1440 columns: (class_of_col == label)
            ohx = spool.tile([P, CK], BF16, tag="ohx")
            nc.vector.tensor_scalar(out=ohx, in0=colck,
                                    scalar1=labels_sb[:, t:t + 1], scalar2=None,
                                    op0=mybir.AluOpType.is_equal)

            # masked = S * onehot  (DVE, PSUM fp32 src -> SBUF bf16)
            masked = spool.tile([P, CK], BF16, tag="masked")
            nc.vector.tensor_mul(masked, s_ps, ohx)

            # T_raw = rowsum(masked); Q_raw = rowsum(masked^2)  (ACT accum)
            nc.scalar.activation(out=junk_bf, in_=masked,
                                 func=mybir.ActivationFunctionType.Copy,
                                 accum_out=t_all[:, t:t + 1])
            nc.scalar.activation(out=junk_bf, in_=masked,
                                 func=mybir.ActivationFunctionType.Square,
                                 accum_out=q_all[:, t:t + 1])

        # ---- phase C: tail over [128, 8] ----
        tp = singles  # small one-off tiles
        norm = tp.tile([P, N_TILES], FP32, tag="norm")
        nc.scalar.activation(out=norm, in_=ss_all,
                             func=mybir.ActivationFunctionType.Sqrt,
                             bias=eps_col)
        rinv = tp.tile([P, N_TILES], FP32, tag="rinv")
        nc.vector.reciprocal(out=rinv, in_=norm)
        tn = tp.tile([P, N_TILES], FP32, tag="tn")
        nc.vector.tensor_mul(tn, t_all, rinv)          # T = T_raw / ||x||
        rinv2 = tp.tile([P, N_TILES], FP32, tag="rinv2")
        nc.vector.tensor_mul(rinv2, rinv, rinv)
        qn = tp.tile([P, N_TILES], FP32, tag="qn")
        nc.vector.tensor_mul(qn, q_all, rinv2)         # Q = Q_raw / ||x||^2

        sd = tp.tile([P, N_TILES], FP32, tag="sd")     # sd = 16 - T
        nc.vector.tensor_scalar(out=sd, in0=tn, scalar1=-1.0, scalar2=float(K),
                                op0=mybir.AluOpType.mult, op1=mybir.AluOpType.add)
        ssq = tp.tile([P, N_TILES], FP32, tag="ssq")   # ssq = 16 - 2T + Q
        nc.vector.tensor_scalar(out=ssq, in0=tn, scalar1=-2.0, scalar2=float(K),
                                op0=mybir.AluOpType.mult, op1=mybir.AluOpType.add)
        nc.vector.tensor_add(ssq, ssq, qn)
        rsd = tp.tile([P, N_TILES], FP32, tag="rsd")
        nc.vector.reciprocal(out=rsd, in_=sd)
        ps = tp.tile([P, N_TILES], FP32, tag="ps")     # per_sample = sd - ssq/sd
        nc.vector.tensor_mul(ps, ssq, rsd)
        nc.vector.tensor_sub(ps, sd, ps)

        nc.sync.dma_start(out=out_dram, in_=ps)

    nc.compile()
    return nc


def get_nc():
    if "nc" not in _NC_CACHE:
        _NC_CACHE["nc"] = _build_nc()
    return _NC_CACHE["nc"]


def _const_inputs():
    ident = np.eye(P, dtype=ml_dtypes.bfloat16)
    colck = np.broadcast_to(
        (np.arange(CK, dtype=np.float32) // K).astype(ml_dtypes.bfloat16),
        (P, CK)).copy()
    return ident, colck


def make_in_maps(x, labels, centers):
    x = np.asarray(x, dtype=np.float32)
    labels = np.asarray(labels)
    centers = np.ascontiguousarray(np.asarray(centers, dtype=np.float32)).reshape(CK, D)
    ident, colck = _const_inputs()
    in_maps = []
    for c in range(N_CORES):
        xs = np.ascontiguousarray(x[c * B_LOCAL:(c + 1) * B_LOCAL])
        ls = labels[c * B_LOCAL:(c + 1) * B_LOCAL]
        ls = np.ascontiguousarray(
            np.asarray(ls).reshape(N_TILES, P).T.astype(np.float32))  # [128, 8]
        in_maps.append({"x": xs, "labels": ls, "centers": centers,
                        "ident": ident, "colck": colck})
    return in_maps


def run(x, labels, centers, trace=False, **kw):
    nc = get_nc()
    in_maps = make_in_maps(x, labels, centers)
    res = bass_utils.run_bass_kernel_spmd(
        nc, in_maps, core_ids=list(range(N_CORES)), trace=trace, **kw)
    total = np.float64(0.0)
    for r in res.results:
        total += np.asarray(r["out"], dtype=np.float64).sum()
    loss = np.float32(total / (N_CORES * B_LOCAL))
    return loss, res


def kernel(x, labels, centers):
    loss, _ = run(x, labels, centers, trace=False)
    return loss
